# revision 1
# baseline (speedup 1.0000x reference)
"""NCC loss (local normalized cross-correlation, window 9^3) on 8 Trainium2
NeuronCores.

Reference: 5 channels [I, J, I^2, J^2, IJ] box-filtered (separable 9-tap mean,
SAME zero-pad) over a 192^3 volume; cc = sigma12^2/(sigma1^2*sigma2^2+eps);
output = 1 - mean(cc).

Sharding: depth axis. Core c computes output slices [24c, 24c+24), reading
padded input slices [24c, 24c+32) of the (+4 both ends) zero-padded volume.
H/W are raw-zero-extended to 200 on the host; prep ops turn the raw zeros
into the correct shifted pad samples, so all filters are pure (unclipped)
banded matmuls and outputs are exactly the interior 192.

Per-core pipeline (validated numerically in model.py, rel err ~6e-5):
  prep   : mean-shift (I-0.5, J-0.5) + products -> 5 bf16 channels/slice,
           [h-part, (ch,w)-free]; h tiles: ext 0..127 and ext 112..199.
  H pass : banded matmuls (TensorE) accumulated over slices into PSUM
           (cumsum over D); bf16 snapshots to SBUF each slice (DVE+ACT).
  D pass : window sum = snapshot difference C[z+8]-C[z-1] (one TT).
  W pass : DMA x-bar transpose of diffs to [w-part, h-free] + banded matmuls.
  cc     : elementwise DVE/ACT; division via exp(ln(num)-ln(den));
           per-partition sums via activation(accum_out).
Host: 1 - sum(partials)/192^3.
"""

import sys

import numpy as np

sys.path.insert(0, "/opt/trn_rl_repo")

import contextlib

import concourse.bacc as bacc
import concourse.mybir as mybir
from concourse import tile
from concourse.bass_utils import run_bass_kernel_spmd

F32 = mybir.dt.float32
BF16 = mybir.dt.bfloat16
AOT = mybir.AluOpType
ACTF = mybir.ActivationFunctionType
AXL = mybir.AxisListType

H = 192
W = 192
D_TOT = 192
HE = 200   # extended h (4 raw-zero pad each side)
WE = 200   # extended w
PAD = 4
N_CORES = 8

HA = 112   # H-pass out: ext rows 4..115  == orig h 0..111
HB = 80    # H-pass out: ext rows 116..195 == orig h 112..191
KT = 128   # chanT partitions: ext-h 0..127
KB = 88    # chanB partitions: ext-h 112..199

BAND_C = 1.0 / 27.0
NCH = 5
FREE = NCH * WE            # 1000 (channel tiles, snapshots)
PIECE = 500                # free elems per matmul (PSUM: placed at 512 offs)
TFREE = NCH * H            # 960 (transposed tiles, F tiles)
TPIECE = 480

EPS = float(np.finfo(np.float32).eps)
TINY = float(np.finfo(np.float32).tiny)


def _band(rows, cols, lo, hi, val):
    k = np.arange(rows)[:, None]
    m = np.arange(cols)[None, :]
    return np.where((k - m >= lo) & (k - m <= hi), val, 0.0).astype(np.float32)


def make_consts():
    import ml_dtypes

    # master upper band, k-m in [0,8]; sliced for all four matmul uses
    return _band(120, 112, 0, 8, BAND_C).astype(ml_dtypes.bfloat16)


def build_program(din, dout):
    assert din == dout + 2 * PAD
    nc = bacc.Bacc(
        "TRN2", target_bir_lowering=False, debug=False, num_devices=N_CORES
    )

    pred_d = nc.dram_tensor("pred", [din, HE, WE], F32, kind="ExternalInput")
    targ_d = nc.dram_tensor("targ", [din, HE, WE], F32, kind="ExternalInput")
    band_d = nc.dram_tensor("band", [120, 112], BF16, kind="ExternalInput")
    out_d = nc.dram_tensor("out", [96, 1], F32, kind="ExternalOutput")

    pred = pred_d.ap()
    targ = targ_d.ap()
    NACC = 2 * dout

    with tile.TileContext(nc) as tc, contextlib.ExitStack() as ctx:
        consts = ctx.enter_context(tc.tile_pool(name="consts", bufs=1))
        raws = ctx.enter_context(tc.tile_pool(name="raws", bufs=3))
        chans = ctx.enter_context(tc.tile_pool(name="chans", bufs=3))
        snaps = ctx.enter_context(tc.tile_pool(name="snaps", bufs=11))
        diffs = ctx.enter_context(tc.tile_pool(name="diffs", bufs=2))
        tts = ctx.enter_context(tc.tile_pool(name="tts", bufs=2))
        fts = ctx.enter_context(tc.tile_pool(name="fts", bufs=2))
        ccs = ctx.enter_context(tc.tile_pool(name="ccs", bufs=2))
        accp = ctx.enter_context(tc.tile_pool(name="accp", bufs=1))
        ps_h = ctx.enter_context(tc.tile_pool(name="psh", bufs=1, space="PSUM"))
        ps_w = ctx.enter_context(tc.tile_pool(name="psw", bufs=1, space="PSUM"))

        band = consts.tile([120, 112], BF16, tag="band")
        nc.sync.dma_start(band[:], band_d.ap())

        bias_nh = consts.tile([128, 1], F32, tag="bias_nh")
        bias_tiny = consts.tile([128, 1], F32, tag="bias_tiny")
        bias_eps = consts.tile([128, 1], F32, tag="bias_eps")
        nc.vector.memset(bias_nh[:], -0.5)
        nc.vector.memset(bias_tiny[:], TINY)
        nc.vector.memset(bias_eps[:], EPS)

        # H-cum PSUM; free padded to 1024 so each 500-piece sits in one bank
        psA = ps_h.tile([HA, 1024], F32, tag="psA")
        psB = ps_h.tile([HB, 1024], F32, tag="psB")
        psA3 = psA.rearrange("p (b w) -> p b w", b=2)  # [*, 2, 512]
        psB3 = psB.rearrange("p (b w) -> p b w", b=2)

        zsnapA = consts.tile([HA, FREE], BF16, tag="zsnapA")
        zsnapB = consts.tile([HB, FREE], BF16, tag="zsnapB")
        nc.vector.memset(zsnapA[:], 0.0)
        nc.vector.memset(zsnapB[:], 0.0)

        acc = accp.tile([96, NACC], F32, tag="acc")
        nc.vector.memset(acc[:], 0.0)

        # Persistent ping-pong diff tiles; free layout [wc:2][ch:5][128] where
        # cols 0..103 of each 128-block hold ext-w 0..103 (wc0) / 96..199
        # (wc1) and cols 104..127 stay zero (memset once) so the x-bar
        # transposes read fully-initialized 128-wide blocks.
        diff_tiles = []
        for pp in range(2):
            dA = diffs.tile(
                [HA, 2 * NCH * 128], BF16, tag=f"dA{pp}", name=f"dA{pp}"
            )
            dB = diffs.tile(
                [HB, 2 * NCH * 128], BF16, tag=f"dB{pp}", name=f"dB{pp}"
            )
            nc.vector.memset(dA[:], 0.0)
            nc.vector.memset(dB[:], 0.0)
            diff_tiles.append((dA, dB))

        snapsA = {}
        snapsB = {}

        def h_pass(z):
            rawT = raws.tile([KT, 2 * WE], F32, tag="rawT", name="rawT")
            rawB = raws.tile([KB, 2 * WE], F32, tag="rawB", name="rawB")
            nc.sync.dma_start(rawT[:, 0:WE], targ[z, 0:KT, :])
            nc.sync.dma_start(rawT[:, WE:], pred[z, 0:KT, :])
            nc.sync.dma_start(rawB[:, 0:WE], targ[z, HE - KB : HE, :])
            nc.sync.dma_start(rawB[:, WE:], pred[z, HE - KB : HE, :])

            chanT = chans.tile([KT, FREE], BF16, tag="chanT", name="chanT")
            chanB = chans.tile([KB, FREE], BF16, tag="chanB", name="chanB")
            for ch, raw in ((chanT, rawT), (chanB, rawB)):
                # ch0 = I-0.5, ch1 = J-0.5
                nc.vector.tensor_scalar_add(ch[:, 0 : 2 * WE], raw[:], -0.5)
                # ch2 = (I-0.5)^2, ch3 = (J-0.5)^2
                nc.scalar.activation(
                    ch[:, 2 * WE : 4 * WE], raw[:], ACTF.Square,
                    bias=bias_nh[0 : ch.shape[0], :],
                )
                # ch4 = (J-0.5)*(I-0.5)
                nc.vector.scalar_tensor_tensor(
                    ch[:, 4 * WE : FREE],
                    raw[:, WE:],
                    -0.5,
                    ch[:, 0:WE],
                    AOT.add,
                    AOT.mult,
                )

            # start only on the first slice (PSUM then accumulates across
            # slices = cumsum over D). stop is a HW no-op; asserting it every
            # slice keeps the simulator's PSUM-read-while-group-open check
            # happy, with skip_group_check for the reopen.
            start = z == 0
            for p in range(2):
                sl = slice(p * PIECE, (p + 1) * PIECE)
                nc.tensor.matmul(
                    psA3[:, p, 0:PIECE], band[0:120, 0:HA], chanT[0:120, sl],
                    start=start, stop=True, skip_group_check=True,
                )
                nc.tensor.matmul(
                    psB3[:, p, 0:PIECE], band[0:KB, 0:HB], chanB[:, sl],
                    start=start, stop=True, skip_group_check=True,
                )

            sA = snaps.tile([HA, FREE], BF16, tag="snapA", name="snapA")
            sB = snaps.tile([HB, FREE], BF16, tag="snapB", name="snapB")
            sA3 = sA.rearrange("p (b w) -> p b w", b=2)
            sB3 = sB.rearrange("p (b w) -> p b w", b=2)
            nc.vector.tensor_copy(sA3[:], psA3[:, :, 0:PIECE])
            nc.scalar.copy(sB3[:], psB3[:, :, 0:PIECE])
            snapsA[z] = sA
            snapsB[z] = sB

        def w_pass(oz):
            hi_A, hi_B = snapsA[oz + 8], snapsB[oz + 8]
            lo_A = zsnapA if oz == 0 else snapsA[oz - 1]
            lo_B = zsnapB if oz == 0 else snapsB[oz - 1]
            snapsA.pop(oz - 2, None)
            snapsB.pop(oz - 2, None)

            # D-filtered slice into the ping-pong diff tiles (valid cols
            # 0..103 per block: wc0 = ext-w 0..103, wc1 = ext-w 96..199)
            dA, dB = diff_tiles[oz % 2]
            for dd, hi, lo in ((dA, hi_A, lo_A), (dB, hi_B, lo_B)):
                d3 = dd.rearrange("p (b c w) -> p b c w", b=2, c=NCH)
                hi3 = hi.rearrange("p (c w) -> p c w", c=NCH)
                lo3 = lo.rearrange("p (c w) -> p c w", c=NCH)
                for wc in range(2):
                    w0 = wc * 96
                    nc.vector.tensor_tensor(
                        d3[:, wc, :, 0:104],
                        hi3[:, :, w0 : w0 + 104],
                        lo3[:, :, w0 : w0 + 104],
                        AOT.subtract,
                    )

            # x-bar transposes: [(HA|HB), 128] -> [128, (HA|HB)] per (wc, ch)
            t0 = tts.tile([128, TFREE], BF16, tag="t0", name="t0")
            t1 = tts.tile([128, TFREE], BF16, tag="t1", name="t1")
            for wc, tt in ((0, t0), (1, t1)):
                for c in range(NCH):
                    src = slice((wc * NCH + c) * 128, (wc * NCH + c + 1) * 128)
                    nc.sync.dma_start_transpose(
                        tt[:, c * H : c * H + HA], dA[:, src]
                    )
                    nc.sync.dma_start_transpose(
                        tt[:, c * H + HA : (c + 1) * H], dB[:, src]
                    )

            pw0 = ps_w.tile([96, 1024], F32, tag="pw0", name="pw0")
            pw1 = ps_w.tile([96, 1024], F32, tag="pw1", name="pw1")
            pw03 = pw0.rearrange("p (b w) -> p b w", b=2)
            pw13 = pw1.rearrange("p (b w) -> p b w", b=2)
            for p in range(2):
                sl = slice(p * TPIECE, (p + 1) * TPIECE)
                nc.tensor.matmul(
                    pw03[:, p, 0:TPIECE], band[0:104, 0:96], t0[0:104, sl],
                    start=True, stop=True,
                )
                nc.tensor.matmul(
                    pw13[:, p, 0:TPIECE], band[0:104, 0:96], t1[0:104, sl],
                    start=True, stop=True,
                )

            f0 = fts.tile([96, TFREE], BF16, tag="f0", name="f0")
            f1 = fts.tile([96, TFREE], BF16, tag="f1", name="f1")
            f03 = f0.rearrange("p (b w) -> p b w", b=2)
            f13 = f1.rearrange("p (b w) -> p b w", b=2)
            nc.vector.tensor_copy(f03[:], pw03[:, :, 0:TPIECE])
            nc.scalar.copy(f13[:], pw13[:, :, 0:TPIECE])

            for fi, ff in enumerate((f0, f1)):
                F_I = ff[:, 0:H]
                F_J = ff[:, H : 2 * H]
                F_IJ = ff[:, 4 * H : 5 * H]

                sc = ccs.tile([96, 6 * H], BF16, tag="sc", name="sc")
                t1_ = sc[:, 0:H]
                s12 = sc[:, H : 2 * H]
                sg1 = sc[:, 2 * H : 3 * H]
                den = sc[:, 4 * H : 5 * H]
                lnf = sc[:, 5 * H : 6 * H]
                sqs = ccs.tile([96, 2 * H], BF16, tag="sqs", name="sqs")
                scf = ccs.tile([96, 3 * H], F32, tag="scf", name="scf")
                s2f = scf[:, 0:H]
                lnn = scf[:, H : 2 * H]
                lnd = scf[:, 2 * H : 3 * H]
                ccout = ccs.tile([96, H], BF16, tag="ccout", name="ccout")

                nc.vector.tensor_tensor(t1_, F_I, F_J, AOT.mult)
                nc.vector.tensor_tensor(s12, F_IJ, t1_, AOT.subtract)
                nc.scalar.activation(s2f, s12, ACTF.Square)
                nc.scalar.activation(sqs[:], ff[:, 0 : 2 * H], ACTF.Square)
                nc.vector.tensor_tensor(
                    sc[:, 2 * H : 4 * H],
                    ff[:, 2 * H : 4 * H],
                    sqs[:],
                    AOT.subtract,
                )
                nc.vector.tensor_tensor(
                    den, sg1, sc[:, 3 * H : 4 * H], AOT.mult
                )
                nc.scalar.activation(lnn, s2f, ACTF.Ln, bias=bias_tiny[0:96, :])
                nc.scalar.activation(lnd, den, ACTF.Ln, bias=bias_eps[0:96, :])
                nc.vector.tensor_tensor(lnf, lnn, lnd, AOT.subtract)
                nc.scalar.activation(
                    ccout[:], lnf, ACTF.Exp,
                    accum_out=acc[:, 2 * oz + fi : 2 * oz + fi + 1],
                )

        for z in range(din):
            h_pass(z)
            oz = z - 8
            if 0 <= oz < dout:
                w_pass(oz)

        accv = accp.tile([96, 1], F32, tag="accv")
        nc.vector.tensor_reduce(accv[:], acc[:], AXL.X, AOT.add)
        nc.sync.dma_start(out_d.ap(), accv[:])

    nc.compile()
    return nc


_PROGRAM_CACHE = {}


def _get_program(din, dout):
    key = (din, dout)
    if key not in _PROGRAM_CACHE:
        _PROGRAM_CACHE[key] = build_program(din, dout)
    return _PROGRAM_CACHE[key]


def kernel(pred, target):
    pred = np.asarray(pred).reshape(D_TOT, H, W).astype(np.float32)
    targ = np.asarray(target).reshape(D_TOT, H, W).astype(np.float32)

    dout = D_TOT // N_CORES
    din = dout + 2 * PAD

    padded_p = np.zeros((D_TOT + 2 * PAD, HE, WE), np.float32)
    padded_t = np.zeros_like(padded_p)
    padded_p[PAD:-PAD, PAD : PAD + H, PAD : PAD + W] = pred
    padded_t[PAD:-PAD, PAD : PAD + H, PAD : PAD + W] = targ

    band = make_consts()
    nc = _get_program(din, dout)

    in_maps = []
    for c in range(N_CORES):
        s = c * dout
        in_maps.append(
            {
                "pred": np.ascontiguousarray(padded_p[s : s + din]),
                "targ": np.ascontiguousarray(padded_t[s : s + din]),
                "band": band,
            }
        )

    res = run_bass_kernel_spmd(nc, in_maps, core_ids=list(range(N_CORES)))
    total = sum(float(r["out"].astype(np.float64).sum()) for r in res.results)
    return np.float32(1.0 - total / float(D_TOT * H * W))



# revision 3
# speedup vs baseline: 3.2383x; 3.2383x over previous
"""NCC loss (local normalized cross-correlation, window 9^3) on 8 Trainium2
NeuronCores — v2 (optimized).

Reference: 5 channels [I, J, I^2, J^2, IJ] box-filtered (separable 9-tap mean,
SAME zero-pad) over a 192^3 volume; cc = sigma12^2/(sigma1^2*sigma2^2+eps);
output = 1 - mean(cc).

Sharding: depth axis. Core c computes output slices [24c, 24c+24), reading
padded input slices [24c, 24c+32) of the (+4 both ends) zero-padded volume.

v2 changes vs baseline (fixes the measured bottlenecks):
  - inputs land as ONE bf16 dram tensor per core [din, 200, 400]
    (targ | pred interleaved on w); loaded with 8 chunked DMAs into two
    resident SBUF tiles instead of 128 per-slice loads.
  - per out-slice, the 20 small DMA transposes (1.23us fixed issue cost
    each, 590us total on the Sync queue = the baseline bottleneck) become
    2 batched x-bar transposes (10 128-col blocks per instruction, 3D
    dst AP), split across the two HWDGE queues (sync + scalar).
  - cc stage: division by exp(ln-ln) (which ping-ponged ACT table sets,
    85 loads x 1.3us) replaced by an int16-magic + 1 Newton-step
    reciprocal on DVE; only Square remains on ACT -> one table set.
    Final accumulation via scalar_tensor_tensor accum_out (no ACT pass).
  - f0/f1 drained into one [96,1920] tile; cc ops run once per out-slice
    on [96,2,192]-strided views (halves DVE/ACT per-op overhead).
  - D-diff: dB pair moved to the (otherwise idle) GPSIMD engine.

Numerically validated in numpy (bf16 inputs, bf16 snapshots, Newton
reciprocal): rel err ~1.5e-5 vs f32 reference.
"""

import sys

import numpy as np

sys.path.insert(0, "/opt/trn_rl_repo")

import contextlib

import concourse.bacc as bacc
import concourse.mybir as mybir
from concourse import tile
from concourse.bass_utils import run_bass_kernel_spmd

F32 = mybir.dt.float32
BF16 = mybir.dt.bfloat16
I16 = mybir.dt.int16
AOT = mybir.AluOpType
ACTF = mybir.ActivationFunctionType
AXL = mybir.AxisListType

H = 192
W = 192
D_TOT = 192
HE = 200   # extended h (4 raw-zero pad each side)
WE = 200   # extended w
PAD = 4
N_CORES = 8

HA = 112   # H-pass out: ext rows 4..115  == orig h 0..111
HB = 80    # H-pass out: ext rows 116..195 == orig h 112..191
KT = 128   # chanT partitions: ext-h 0..127
KB = 88    # chanB partitions: ext-h 112..199

BAND_C = 1.0 / 27.0
NCH = 5
FREE = NCH * WE            # 1000 (channel tiles, snapshots)
PIECE = 500                # free elems per matmul (PSUM: placed at 512 offs)
TPIECE = 480

EPS = float(np.finfo(np.float32).eps)
MAGIC = 0x7EF0             # bf16 reciprocal seed: bits(r0) = MAGIC - bits(x)


def _band(rows, cols, lo, hi, val):
    k = np.arange(rows)[:, None]
    m = np.arange(cols)[None, :]
    return np.where((k - m >= lo) & (k - m <= hi), val, 0.0).astype(np.float32)


def make_consts():
    import ml_dtypes

    # master upper band, k-m in [0,8]; sliced for all four matmul uses
    return _band(120, 112, 0, 8, BAND_C).astype(ml_dtypes.bfloat16)


def build_program(din, dout):
    assert din == dout + 2 * PAD
    nc = bacc.Bacc(
        "TRN2", target_bir_lowering=False, debug=False, num_devices=N_CORES
    )

    xin_d = nc.dram_tensor("xin", [din, HE, 2 * WE], BF16, kind="ExternalInput")
    band_d = nc.dram_tensor("band", [120, 112], BF16, kind="ExternalInput")
    out_d = nc.dram_tensor("out", [96, 1], F32, kind="ExternalOutput")

    xin = xin_d.ap()

    with tile.TileContext(nc) as tc, contextlib.ExitStack() as ctx:
        consts = ctx.enter_context(tc.tile_pool(name="consts", bufs=1))
        bigx = ctx.enter_context(tc.tile_pool(name="bigx", bufs=1))
        chans = ctx.enter_context(tc.tile_pool(name="chans", bufs=3))
        snaps = ctx.enter_context(tc.tile_pool(name="snaps", bufs=11))
        diffs = ctx.enter_context(tc.tile_pool(name="diffs", bufs=2))
        tts = ctx.enter_context(tc.tile_pool(name="tts", bufs=2))
        ffs = ctx.enter_context(tc.tile_pool(name="ffs", bufs=2))
        ccs = ctx.enter_context(tc.tile_pool(name="ccs", bufs=2))
        accp = ctx.enter_context(tc.tile_pool(name="accp", bufs=1))
        ps_h = ctx.enter_context(tc.tile_pool(name="psh", bufs=1, space="PSUM"))
        ps_w = ctx.enter_context(tc.tile_pool(name="psw", bufs=1, space="PSUM"))

        band = consts.tile([120, 112], BF16, tag="band")
        nc.sync.dma_start(band[:], band_d.ap())

        bias_nh = consts.tile([128, 1], F32, tag="bias_nh")
        nc.vector.memset(bias_nh[:], -0.5)

        # resident input: rows 0..127 (T) and 112..199 (B), one z-slice =
        # 400 bf16 per partition ([0:200]=targ, [200:400]=pred)
        XT = bigx.tile([KT, din * 2 * WE], BF16, tag="XT")
        XB = bigx.tile([KB, din * 2 * WE], BF16, tag="XB")
        XT3 = XT.rearrange("p (z w) -> p z w", z=din)
        XB3 = XB.rearrange("p (z w) -> p z w", z=din)
        CH = 8  # slices per input-load chunk
        for c in range(din // CH):
            z0 = c * CH
            src = xin[z0 : z0 + CH, :, :]
            nc.scalar.dma_start(
                XT3[:, z0 : z0 + CH, :],
                src[:, 0:KT, :].rearrange("z h w -> h z w"),
            )
            nc.scalar.dma_start(
                XB3[:, z0 : z0 + CH, :],
                src[:, HE - KB : HE, :].rearrange("z h w -> h z w"),
            )

        # H-cum PSUM; free padded to 1024 so each 500-piece sits in one bank
        psA = ps_h.tile([HA, 1024], F32, tag="psA")
        psB = ps_h.tile([HB, 1024], F32, tag="psB")
        psA3 = psA.rearrange("p (b w) -> p b w", b=2)  # [*, 2, 512]
        psB3 = psB.rearrange("p (b w) -> p b w", b=2)

        zsnapA = consts.tile([HA, FREE], BF16, tag="zsnapA")
        zsnapB = consts.tile([HB, FREE], BF16, tag="zsnapB")
        nc.vector.memset(zsnapA[:], 0.0)
        nc.vector.memset(zsnapB[:], 0.0)

        acc = accp.tile([96, dout], F32, tag="acc")
        nc.vector.memset(acc[:], 0.0)

        # Persistent ping-pong diff tiles; free layout [wc:2][ch:5][128] where
        # cols 0..103 of each 128-block hold ext-w 0..103 (wc0) / 96..199
        # (wc1) and cols 104..127 stay zero (memset once) so the x-bar
        # transposes read fully-initialized 128-wide blocks.
        diff_tiles = []
        for pp in range(2):
            dA = diffs.tile(
                [HA, 2 * NCH * 128], BF16, tag=f"dA{pp}", name=f"dA{pp}"
            )
            dB = diffs.tile(
                [HB, 2 * NCH * 128], BF16, tag=f"dB{pp}", name=f"dB{pp}"
            )
            nc.vector.memset(dA[:], 0.0)
            nc.vector.memset(dB[:], 0.0)
            diff_tiles.append((dA, dB))

        snapsA = {}
        snapsB = {}

        def h_pass(z):
            rawT = XT3[:, z, :]   # [128, 400] bf16
            rawB = XB3[:, z, :]   # [88, 400]

            chanT = chans.tile([KT, FREE], BF16, tag="chanT", name="chanT")
            chanB = chans.tile([KB, FREE], BF16, tag="chanB", name="chanB")
            for ch, raw in ((chanT, rawT), (chanB, rawB)):
                np_ = ch.shape[0]
                # ch0 = I-0.5, ch1 = J-0.5
                nc.vector.tensor_scalar_add(ch[:, 0 : 2 * WE], raw, -0.5)
                # ch2 = (I-0.5)^2, ch3 = (J-0.5)^2
                nc.scalar.activation(
                    ch[:, 2 * WE : 4 * WE], raw, ACTF.Square,
                    bias=bias_nh[0:np_, :],
                )
                # ch4 = (J-0.5)*(I-0.5)
                nc.vector.scalar_tensor_tensor(
                    ch[:, 4 * WE : FREE],
                    raw[:, WE : 2 * WE],
                    -0.5,
                    ch[:, 0:WE],
                    AOT.add,
                    AOT.mult,
                )

            # start only on the first slice (PSUM then accumulates across
            # slices = cumsum over D). stop is a HW no-op; asserting it every
            # slice keeps the simulator's PSUM-read-while-group-open check
            # happy, with skip_group_check for the reopen.
            start = z == 0
            # A pieces back-to-back (shared lhsT), then B
            for p in range(2):
                sl = slice(p * PIECE, (p + 1) * PIECE)
                nc.tensor.matmul(
                    psA3[:, p, 0:PIECE], band[0:120, 0:HA], chanT[0:120, sl],
                    start=start, stop=True, skip_group_check=True,
                )
            for p in range(2):
                sl = slice(p * PIECE, (p + 1) * PIECE)
                nc.tensor.matmul(
                    psB3[:, p, 0:PIECE], band[0:KB, 0:HB], chanB[:, sl],
                    start=start, stop=True, skip_group_check=True,
                )

            sA = snaps.tile([HA, FREE], BF16, tag="snapA", name="snapA")
            sB = snaps.tile([HB, FREE], BF16, tag="snapB", name="snapB")
            sA3 = sA.rearrange("p (b w) -> p b w", b=2)
            sB3 = sB.rearrange("p (b w) -> p b w", b=2)
            nc.vector.tensor_copy(sA3[:], psA3[:, :, 0:PIECE])
            nc.scalar.copy(sB3[:], psB3[:, :, 0:PIECE])
            snapsA[z] = sA
            snapsB[z] = sB

        def w_pass(oz):
            hi_A, hi_B = snapsA[oz + 8], snapsB[oz + 8]
            lo_A = zsnapA if oz == 0 else snapsA[oz - 1]
            lo_B = zsnapB if oz == 0 else snapsB[oz - 1]
            snapsA.pop(oz - 2, None)
            snapsB.pop(oz - 2, None)

            # D-filtered slice into the ping-pong diff tiles (valid cols
            # 0..103 per block: wc0 = ext-w 0..103, wc1 = ext-w 96..199).
            # dA pair on DVE, dB pair on GPSIMD (otherwise idle).
            dA, dB = diff_tiles[oz % 2]
            for dd, hi, lo, eng in (
                (dA, hi_A, lo_A, nc.vector),
                (dB, hi_B, lo_B, nc.vector),
            ):
                d3 = dd.rearrange("p (b c w) -> p b c w", b=2, c=NCH)
                hi3 = hi.rearrange("p (c w) -> p c w", c=NCH)
                lo3 = lo.rearrange("p (c w) -> p c w", c=NCH)
                for wc in range(2):
                    w0 = wc * 96
                    eng.tensor_tensor(
                        d3[:, wc, :, 0:104],
                        hi3[:, :, w0 : w0 + 104],
                        lo3[:, :, w0 : w0 + 104],
                        AOT.subtract,
                    )

            # batched x-bar transposes: all 10 (wc,ch) 128-blocks of each
            # diff tile in ONE instruction; block b lands at free offset
            # b*192 (+0 for A-rows 0..111, +112 for B-rows 112..191).
            tt = tts.tile([128, 2 * NCH * H], BF16, tag="tt", name="tt")
            tt3 = tt.rearrange("p (b h) -> p b h", b=2 * NCH)
            nc.sync.dma_start_transpose(tt3[:, :, 0:HA], dA[:])
            nc.scalar.dma_start_transpose(tt3[:, :, HA:H], dB[:])

            pw0 = ps_w.tile([96, 1024], F32, tag="pw0", name="pw0")
            pw1 = ps_w.tile([96, 1024], F32, tag="pw1", name="pw1")
            pw03 = pw0.rearrange("p (b w) -> p b w", b=2)
            pw13 = pw1.rearrange("p (b w) -> p b w", b=2)
            for p in range(2):
                sl = slice(p * TPIECE, (p + 1) * TPIECE)
                sl1 = slice(NCH * H + p * TPIECE, NCH * H + (p + 1) * TPIECE)
                nc.tensor.matmul(
                    pw03[:, p, 0:TPIECE], band[0:104, 0:96], tt[0:104, sl],
                    start=True, stop=True,
                )
                nc.tensor.matmul(
                    pw13[:, p, 0:TPIECE], band[0:104, 0:96], tt[0:104, sl1],
                    start=True, stop=True,
                )

            # drain both wc halves into ONE [96, 1920] tile; cc ops then run
            # once per out-slice on [96, 2, 192]-strided 3D views.
            ff = ffs.tile([96, 2 * NCH * H], BF16, tag="ff", name="ff")
            ff3 = ff.rearrange("p (b w) -> p b w", b=2)  # [96, 2, 960]
            nc.vector.tensor_copy(
                ff3[:, 0:1, :].rearrange("p o (b w) -> p (o b) w", b=2),
                pw03[:, :, 0:TPIECE],
            )
            nc.scalar.copy(
                ff3[:, 1:2, :].rearrange("p o (b w) -> p (o b) w", b=2),
                pw13[:, :, 0:TPIECE],
            )

            F_I = ff3[:, :, 0:H]
            F_J = ff3[:, :, H : 2 * H]
            F_SQ = ff3[:, :, 0 : 2 * H]          # [I, J] pair
            F_CONV = ff3[:, :, 2 * H : 4 * H]    # [conv_I2, conv_J2]
            F_IJ = ff3[:, :, 4 * H : 5 * H]

            sc = ccs.tile([96, 2 * 1152], BF16, tag="sc", name="sc")
            sc3 = sc.rearrange("p (b w) -> p b w", b=2)
            t1v = sc3[:, :, 0:H]
            s12 = sc3[:, :, H : 2 * H]
            sqs = sc3[:, :, 2 * H : 4 * H]
            sg = sc3[:, :, 4 * H : 6 * H]
            sg1 = sc3[:, :, 4 * H : 5 * H]
            sg2 = sc3[:, :, 5 * H : 6 * H]
            scd = ccs.tile([96, 2 * 960], BF16, tag="scd", name="scd")
            scd3 = scd.rearrange("p (b w) -> p b w", b=2)
            den = scd3[:, :, 0:H]
            r0 = scd3[:, :, H : 2 * H]
            tq = scd3[:, :, 2 * H : 3 * H]
            pq = scd3[:, :, 3 * H : 4 * H]
            r1 = scd3[:, :, 4 * H : 5 * H]
            den2 = t1v   # t1v dead after s12
            s2f = tq     # tq dead after pq
            ccout = pq   # pq dead after r1

            nc.vector.tensor_tensor(t1v, F_I, F_J, AOT.mult)
            nc.vector.tensor_tensor(s12, F_IJ, t1v, AOT.subtract)
            nc.scalar.activation(sqs, F_SQ, ACTF.Square)
            nc.vector.tensor_tensor(sg, F_CONV, sqs, AOT.subtract)
            nc.vector.tensor_tensor(den, sg1, sg2, AOT.mult)
            nc.vector.tensor_scalar_max(den2, den, EPS)
            # reciprocal seed: bits(r0) = MAGIC - bits(den2)
            nc.vector.tensor_scalar(
                r0.bitcast(I16), den2.bitcast(I16), -1, MAGIC,
                AOT.mult, AOT.add,
            )
            # one Newton step: r1 = r0*(2 - den2*r0)
            nc.vector.tensor_tensor(tq, den2, r0, AOT.mult)
            nc.vector.tensor_tensor(pq, r0, tq, AOT.mult)
            nc.vector.scalar_tensor_tensor(
                r1, r0, 2.0, pq, AOT.mult, AOT.subtract
            )
            nc.scalar.activation(s2f, s12, ACTF.Square)
            # cc = s12^2 * recip(den), accumulated per-partition into acc
            nc.vector.scalar_tensor_tensor(
                ccout, s2f, 1.0, r1, AOT.mult, AOT.mult,
                accum_out=acc[:, oz : oz + 1],
            )

        for z in range(din):
            h_pass(z)
            oz = z - 8
            if 0 <= oz < dout:
                w_pass(oz)

        accv = accp.tile([96, 1], F32, tag="accv")
        nc.vector.tensor_reduce(accv[:], acc[:], AXL.X, AOT.add)
        nc.sync.dma_start(out_d.ap(), accv[:])

    nc.compile()
    return nc


_PROGRAM_CACHE = {}


def _get_program(din, dout):
    key = (din, dout)
    if key not in _PROGRAM_CACHE:
        _PROGRAM_CACHE[key] = build_program(din, dout)
    return _PROGRAM_CACHE[key]


def make_in_maps(pred, target):
    import ml_dtypes

    pred = np.asarray(pred).reshape(D_TOT, H, W).astype(np.float32)
    targ = np.asarray(target).reshape(D_TOT, H, W).astype(np.float32)

    dout = D_TOT // N_CORES
    din = dout + 2 * PAD

    # one interleaved, padded, bf16 volume: [D+8, 200, 400]
    big = np.zeros((D_TOT + 2 * PAD, HE, 2 * WE), ml_dtypes.bfloat16)
    big[PAD:-PAD, PAD : PAD + H, PAD : PAD + W] = targ
    big[PAD:-PAD, PAD : PAD + H, WE + PAD : WE + PAD + W] = pred

    band = make_consts()
    in_maps = []
    for c in range(N_CORES):
        s = c * dout
        in_maps.append(
            {
                "xin": np.ascontiguousarray(big[s : s + din]),
                "band": band,
            }
        )
    return in_maps, din, dout


def kernel(pred, target):
    in_maps, din, dout = make_in_maps(pred, target)
    nc = _get_program(din, dout)
    res = run_bass_kernel_spmd(nc, in_maps, core_ids=list(range(N_CORES)))
    total = sum(float(r["out"].astype(np.float64).sum()) for r in res.results)
    return np.float32(1.0 - total / float(D_TOT * H * W))


# revision 8
# speedup vs baseline: 3.6703x; 1.1334x over previous
"""NCC loss (local normalized cross-correlation, window 9^3) on 8 Trainium2
NeuronCores — v2 (optimized).

Reference: 5 channels [I, J, I^2, J^2, IJ] box-filtered (separable 9-tap mean,
SAME zero-pad) over a 192^3 volume; cc = sigma12^2/(sigma1^2*sigma2^2+eps);
output = 1 - mean(cc).

Sharding: depth axis. Core c computes output slices [24c, 24c+24), reading
padded input slices [24c, 24c+32) of the (+4 both ends) zero-padded volume.

v2 changes vs baseline (fixes the measured bottlenecks):
  - inputs land as ONE bf16 dram tensor per core [din, 200, 400]
    (targ | pred interleaved on w); loaded with 8 chunked DMAs into two
    resident SBUF tiles instead of 128 per-slice loads.
  - per out-slice, the 20 small DMA transposes (1.23us fixed issue cost
    each, 590us total on the Sync queue = the baseline bottleneck) become
    2 batched x-bar transposes (10 128-col blocks per instruction, 3D
    dst AP), split across the two HWDGE queues (sync + scalar).
  - cc stage: division by exp(ln-ln) (which ping-ponged ACT table sets,
    85 loads x 1.3us) replaced by an int16-magic + 1 Newton-step
    reciprocal on DVE; only Square remains on ACT -> one table set.
    Final accumulation via scalar_tensor_tensor accum_out (no ACT pass).
  - f0/f1 drained into one [96,1920] tile; cc ops run once per out-slice
    on [96,2,192]-strided views (halves DVE/ACT per-op overhead).
  - D-diff: dB pair moved to the (otherwise idle) GPSIMD engine.

Numerically validated in numpy (bf16 inputs, bf16 snapshots, Newton
reciprocal): rel err ~1.5e-5 vs f32 reference.
"""

import sys

import numpy as np

sys.path.insert(0, "/opt/trn_rl_repo")

import contextlib

import concourse.bacc as bacc
import concourse.mybir as mybir
from concourse import tile
from concourse.bass_utils import run_bass_kernel_spmd

F32 = mybir.dt.float32
BF16 = mybir.dt.bfloat16
I16 = mybir.dt.int16
AOT = mybir.AluOpType
ACTF = mybir.ActivationFunctionType
AXL = mybir.AxisListType

H = 192
W = 192
D_TOT = 192
HE = 200   # extended h (4 raw-zero pad each side)
WE = 200   # extended w
PAD = 4
N_CORES = 8

HA = 112   # H-pass out: ext rows 4..115  == orig h 0..111
HB = 80    # H-pass out: ext rows 116..195 == orig h 112..191
KT = 128   # chanT partitions: ext-h 0..127
KB = 88    # chanB partitions: ext-h 112..199

BAND_C = 1.0 / 27.0
NCH = 5
FREE = NCH * WE            # 1000 (channel tiles, snapshots)
PIECE = 500                # free elems per matmul (PSUM: placed at 512 offs)
TPIECE = 480

EPS = float(np.finfo(np.float32).eps)
MAGIC = 0x7EF0             # bf16 reciprocal seed: bits(r0) = MAGIC - bits(x)


def _band(rows, cols, lo, hi, val):
    k = np.arange(rows)[:, None]
    m = np.arange(cols)[None, :]
    return np.where((k - m >= lo) & (k - m <= hi), val, 0.0).astype(np.float32)


def make_consts():
    import ml_dtypes

    # master upper band, k-m in [0,8]; sliced for all four matmul uses
    return _band(120, 112, 0, 8, BAND_C).astype(ml_dtypes.bfloat16)


def build_program(din, dout):
    assert din == dout + 2 * PAD
    nc = bacc.Bacc(
        "TRN2", target_bir_lowering=False, debug=False, num_devices=N_CORES
    )

    xin_d = nc.dram_tensor("xin", [din, HE, 2 * WE], BF16, kind="ExternalInput")
    band_d = nc.dram_tensor("band", [120, 112], BF16, kind="ExternalInput")
    out_d = nc.dram_tensor("out", [96, 1], F32, kind="ExternalOutput")

    xin = xin_d.ap()

    with tile.TileContext(nc) as tc, contextlib.ExitStack() as ctx:
        consts = ctx.enter_context(tc.tile_pool(name="consts", bufs=1))
        bigx = ctx.enter_context(tc.tile_pool(name="bigx", bufs=1))
        chans = ctx.enter_context(tc.tile_pool(name="chans", bufs=3))
        snaps = ctx.enter_context(tc.tile_pool(name="snaps", bufs=11))
        diffs = ctx.enter_context(tc.tile_pool(name="diffs", bufs=2))
        tts = ctx.enter_context(tc.tile_pool(name="tts", bufs=2))
        ffs = ctx.enter_context(tc.tile_pool(name="ffs", bufs=2))
        ccs = ctx.enter_context(tc.tile_pool(name="ccs", bufs=2))
        accp = ctx.enter_context(tc.tile_pool(name="accp", bufs=1))
        ps_h = ctx.enter_context(tc.tile_pool(name="psh", bufs=1, space="PSUM"))
        ps_w = ctx.enter_context(tc.tile_pool(name="psw", bufs=1, space="PSUM"))

        band = consts.tile([120, 112], BF16, tag="band")
        nc.sync.dma_start(band[:], band_d.ap())

        bias_nh = consts.tile([128, 1], F32, tag="bias_nh")
        nc.vector.memset(bias_nh[:], -0.5)

        # resident input: rows 0..127 (T) and 112..199 (B), one z-slice =
        # 400 bf16 per partition ([0:200]=targ, [200:400]=pred)
        XT = bigx.tile([KT, din * 2 * WE], BF16, tag="XT")
        XB = bigx.tile([KB, din * 2 * WE], BF16, tag="XB")
        XT3 = XT.rearrange("p (z w) -> p z w", z=din)
        XB3 = XB.rearrange("p (z w) -> p z w", z=din)
        CH = 8  # slices per input-load chunk
        for c in range(din // CH):
            z0 = c * CH
            src = xin[z0 : z0 + CH, :, :]
            nc.sync.dma_start(
                XT3[:, z0 : z0 + CH, :],
                src[:, 0:KT, :].rearrange("z h w -> h z w"),
            )
            nc.sync.dma_start(
                XB3[:, z0 : z0 + CH, :],
                src[:, HE - KB : HE, :].rearrange("z h w -> h z w"),
            )

        # H-cum PSUM; free padded to 1024 so each 500-piece sits in one bank
        psA = ps_h.tile([HA, 1024], F32, tag="psA")
        psB = ps_h.tile([HB, 1024], F32, tag="psB")
        psA3 = psA.rearrange("p (b w) -> p b w", b=2)  # [*, 2, 512]
        psB3 = psB.rearrange("p (b w) -> p b w", b=2)

        zsnapA = consts.tile([HA, FREE], BF16, tag="zsnapA")
        zsnapB = consts.tile([HB, FREE], BF16, tag="zsnapB")
        nc.vector.memset(zsnapA[:], 0.0)
        nc.vector.memset(zsnapB[:], 0.0)

        acc = accp.tile([96, dout], F32, tag="acc")
        nc.vector.memset(acc[:], 0.0)

        # Persistent ping-pong diff tiles; free layout [wc:2][ch:5][128] where
        # cols 0..103 of each 128-block hold ext-w 0..103 (wc0) / 96..199
        # (wc1) and cols 104..127 stay zero (memset once) so the x-bar
        # transposes read fully-initialized 128-wide blocks.
        diff_tiles = []
        for pp in range(2):
            dA = diffs.tile(
                [HA, 2 * NCH * 128], BF16, tag=f"dA{pp}", name=f"dA{pp}"
            )
            dB = diffs.tile(
                [HB, 2 * NCH * 128], BF16, tag=f"dB{pp}", name=f"dB{pp}"
            )
            nc.vector.memset(dA[:], 0.0)
            nc.vector.memset(dB[:], 0.0)
            diff_tiles.append((dA, dB))

        snapsA = {}
        snapsB = {}

        def h_pass(z):
            rawT = XT3[:, z, :]   # [128, 400] bf16
            rawB = XB3[:, z, :]   # [88, 400]

            chanT = chans.tile([KT, FREE], BF16, tag="chanT", name="chanT")
            chanB = chans.tile([KB, FREE], BF16, tag="chanB", name="chanB")
            for ch, raw in ((chanT, rawT), (chanB, rawB)):
                np_ = ch.shape[0]
                # ch0 = I-0.5, ch1 = J-0.5
                nc.vector.tensor_scalar_add(ch[:, 0 : 2 * WE], raw, -0.5)
                # ch2 = (I-0.5)^2, ch3 = (J-0.5)^2
                nc.scalar.activation(
                    ch[:, 2 * WE : 4 * WE], raw, ACTF.Square,
                    bias=bias_nh[0:np_, :],
                )
                # ch4 = (J-0.5)*(I-0.5)
                nc.vector.scalar_tensor_tensor(
                    ch[:, 4 * WE : FREE],
                    raw[:, WE : 2 * WE],
                    -0.5,
                    ch[:, 0:WE],
                    AOT.add,
                    AOT.mult,
                )

            # start only on the first slice (PSUM then accumulates across
            # slices = cumsum over D). stop is a HW no-op; asserting it every
            # slice keeps the simulator's PSUM-read-while-group-open check
            # happy, with skip_group_check for the reopen.
            start = z == 0
            # A pieces back-to-back (shared lhsT), then B
            for p in range(2):
                sl = slice(p * PIECE, (p + 1) * PIECE)
                nc.tensor.matmul(
                    psA3[:, p, 0:PIECE], band[0:120, 0:HA], chanT[0:120, sl],
                    start=start, stop=True, skip_group_check=True,
                )
            for p in range(2):
                sl = slice(p * PIECE, (p + 1) * PIECE)
                nc.tensor.matmul(
                    psB3[:, p, 0:PIECE], band[0:KB, 0:HB], chanB[:, sl],
                    start=start, stop=True, skip_group_check=True,
                )

            sA = snaps.tile([HA, FREE], BF16, tag="snapA", name="snapA")
            sB = snaps.tile([HB, FREE], BF16, tag="snapB", name="snapB")
            sA3 = sA.rearrange("p (b w) -> p b w", b=2)
            sB3 = sB.rearrange("p (b w) -> p b w", b=2)
            nc.vector.tensor_copy(sA3[:], psA3[:, :, 0:PIECE])
            nc.scalar.copy(sB3[:], psB3[:, :, 0:PIECE])
            snapsA[z] = sA
            snapsB[z] = sB

        def w_pass(oz):
            hi_A, hi_B = snapsA[oz + 8], snapsB[oz + 8]
            lo_A = zsnapA if oz == 0 else snapsA[oz - 1]
            lo_B = zsnapB if oz == 0 else snapsB[oz - 1]
            snapsA.pop(oz - 2, None)
            snapsB.pop(oz - 2, None)

            # D-filtered slice into the ping-pong diff tiles (valid cols
            # 0..103 per block: wc0 = ext-w 0..103, wc1 = ext-w 96..199).
            # dA pair on DVE, dB pair on GPSIMD (otherwise idle).
            dA, dB = diff_tiles[oz % 2]
            for dd, hi, lo, eng in (
                (dA, hi_A, lo_A, nc.vector),
                (dB, hi_B, lo_B, nc.vector),
            ):
                d3 = dd.rearrange("p (b c w) -> p b c w", b=2, c=NCH)
                hi3 = hi.rearrange("p (c w) -> p c w", c=NCH)
                lo3 = lo.rearrange("p (c w) -> p c w", c=NCH)
                for wc in range(2):
                    w0 = wc * 96
                    eng.tensor_tensor(
                        d3[:, wc, :, 0:104],
                        hi3[:, :, w0 : w0 + 104],
                        lo3[:, :, w0 : w0 + 104],
                        AOT.subtract,
                    )

            # batched x-bar transposes: all 10 (wc,ch) 128-blocks of each
            # diff tile in ONE instruction; block b lands at free offset
            # b*192 (+0 for A-rows 0..111, +112 for B-rows 112..191).
            tt = tts.tile([128, 2 * NCH * H], BF16, tag="tt", name="tt")
            tt3 = tt.rearrange("p (b h) -> p b h", b=2 * NCH)
            nc.sync.dma_start_transpose(tt3[:, :, 0:HA], dA[:])
            nc.sync.dma_start_transpose(tt3[:, :, HA:H], dB[:])

            pw0 = ps_w.tile([96, 1024], F32, tag="pw0", name="pw0")
            pw1 = ps_w.tile([96, 1024], F32, tag="pw1", name="pw1")
            pw03 = pw0.rearrange("p (b w) -> p b w", b=2)
            pw13 = pw1.rearrange("p (b w) -> p b w", b=2)
            for p in range(2):
                sl = slice(p * TPIECE, (p + 1) * TPIECE)
                sl1 = slice(NCH * H + p * TPIECE, NCH * H + (p + 1) * TPIECE)
                nc.tensor.matmul(
                    pw03[:, p, 0:TPIECE], band[0:104, 0:96], tt[0:104, sl],
                    start=True, stop=True,
                )
                nc.tensor.matmul(
                    pw13[:, p, 0:TPIECE], band[0:104, 0:96], tt[0:104, sl1],
                    start=True, stop=True,
                )

            # drain both wc halves into ONE [96, 1920] tile; cc ops then run
            # once per out-slice on [96, 2, 192]-strided 3D views.
            ff = ffs.tile([96, 2 * NCH * H], BF16, tag="ff", name="ff")
            ff3 = ff.rearrange("p (b w) -> p b w", b=2)  # [96, 2, 960]
            nc.scalar.copy(
                ff3[:, 0:1, :].rearrange("p o (b w) -> p (o b) w", b=2),
                pw03[:, :, 0:TPIECE],
            )
            nc.scalar.copy(
                ff3[:, 1:2, :].rearrange("p o (b w) -> p (o b) w", b=2),
                pw13[:, :, 0:TPIECE],
            )

            F_I = ff3[:, :, 0:H]
            F_J = ff3[:, :, H : 2 * H]
            F_SQ = ff3[:, :, 0 : 2 * H]          # [I, J] pair
            F_CONV = ff3[:, :, 2 * H : 4 * H]    # [conv_I2, conv_J2]
            F_IJ = ff3[:, :, 4 * H : 5 * H]

            sc = ccs.tile([96, 2 * 1152], BF16, tag="sc", name="sc")
            sc3 = sc.rearrange("p (b w) -> p b w", b=2)
            t1v = sc3[:, :, 0:H]
            s12 = sc3[:, :, H : 2 * H]
            sqs = sc3[:, :, 2 * H : 4 * H]
            sg = sc3[:, :, 4 * H : 6 * H]
            sg1 = sc3[:, :, 4 * H : 5 * H]
            sg2 = sc3[:, :, 5 * H : 6 * H]
            scd = ccs.tile([96, 2 * 960], BF16, tag="scd", name="scd")
            scd3 = scd.rearrange("p (b w) -> p b w", b=2)
            den = scd3[:, :, 0:H]
            r0 = scd3[:, :, H : 2 * H]
            tq = scd3[:, :, 2 * H : 3 * H]
            r1n = scd3[:, :, 3 * H : 4 * H]
            s2f = scd3[:, :, 4 * H : 5 * H]
            den2 = t1v   # t1v dead after s12
            ccout = tq   # tq dead after r1n

            nc.vector.tensor_tensor(t1v, F_I, F_J, AOT.mult)
            nc.vector.tensor_tensor(s12, F_IJ, t1v, AOT.subtract)
            nc.scalar.activation(sqs, F_SQ, ACTF.Square)
            nc.vector.tensor_tensor(sg, F_CONV, sqs, AOT.subtract)
            nc.vector.tensor_tensor(den, sg1, sg2, AOT.mult)
            nc.vector.tensor_scalar_max(den2, den, EPS)
            # reciprocal seed: bits(r0) = MAGIC - bits(den2)
            nc.vector.tensor_scalar(
                r0.bitcast(I16), den2.bitcast(I16), -1, MAGIC,
                AOT.mult, AOT.add,
            )
            # one Newton step, sign-folded: r1n = (den2*r0 - 2)*r0 = -recip
            nc.vector.tensor_tensor(tq, den2, r0, AOT.mult)
            nc.vector.scalar_tensor_tensor(
                r1n, tq, 2.0, r0, AOT.subtract, AOT.mult
            )
            nc.scalar.activation(s2f, s12, ACTF.Square)
            # cc = (-s2f) * r1n = s12^2 * recip(den), accumulated into acc
            nc.vector.scalar_tensor_tensor(
                ccout, s2f, -1.0, r1n, AOT.mult, AOT.mult,
                accum_out=acc[:, oz : oz + 1],
            )

        for z in range(din):
            h_pass(z)
            oz = z - 8
            if 0 <= oz < dout:
                w_pass(oz)

        accv = accp.tile([96, 1], F32, tag="accv")
        nc.vector.tensor_reduce(accv[:], acc[:], AXL.X, AOT.add)
        nc.sync.dma_start(out_d.ap(), accv[:])

    nc.compile()
    return nc


_PROGRAM_CACHE = {}


def _get_program(din, dout):
    key = (din, dout)
    if key not in _PROGRAM_CACHE:
        _PROGRAM_CACHE[key] = build_program(din, dout)
    return _PROGRAM_CACHE[key]


def make_in_maps(pred, target):
    import ml_dtypes

    pred = np.asarray(pred).reshape(D_TOT, H, W).astype(np.float32)
    targ = np.asarray(target).reshape(D_TOT, H, W).astype(np.float32)

    dout = D_TOT // N_CORES
    din = dout + 2 * PAD

    # one interleaved, padded, bf16 volume: [D+8, 200, 400]
    big = np.zeros((D_TOT + 2 * PAD, HE, 2 * WE), ml_dtypes.bfloat16)
    big[PAD:-PAD, PAD : PAD + H, PAD : PAD + W] = targ
    big[PAD:-PAD, PAD : PAD + H, WE + PAD : WE + PAD + W] = pred

    band = make_consts()
    in_maps = []
    for c in range(N_CORES):
        s = c * dout
        in_maps.append(
            {
                "xin": np.ascontiguousarray(big[s : s + din]),
                "band": band,
            }
        )
    return in_maps, din, dout


def kernel(pred, target):
    in_maps, din, dout = make_in_maps(pred, target)
    nc = _get_program(din, dout)
    res = run_bass_kernel_spmd(nc, in_maps, core_ids=list(range(N_CORES)))
    total = sum(float(r["out"].astype(np.float64).sum()) for r in res.results)
    return np.float32(1.0 - total / float(D_TOT * H * W))


# revision 10
# speedup vs baseline: 3.8078x; 1.0375x over previous
"""NCC loss (local normalized cross-correlation, window 9^3) on 8 Trainium2
NeuronCores — v2 (optimized).

Reference: 5 channels [I, J, I^2, J^2, IJ] box-filtered (separable 9-tap mean,
SAME zero-pad) over a 192^3 volume; cc = sigma12^2/(sigma1^2*sigma2^2+eps);
output = 1 - mean(cc).

Sharding: depth axis. Core c computes output slices [24c, 24c+24), reading
padded input slices [24c, 24c+32) of the (+4 both ends) zero-padded volume.

v2 changes vs baseline (fixes the measured bottlenecks):
  - inputs land as ONE bf16 dram tensor per core [din, 200, 400]
    (targ | pred interleaved on w); loaded with 8 chunked DMAs into two
    resident SBUF tiles instead of 128 per-slice loads.
  - per out-slice, the 20 small DMA transposes (1.23us fixed issue cost
    each, 590us total on the Sync queue = the baseline bottleneck) become
    2 batched x-bar transposes (10 128-col blocks per instruction, 3D
    dst AP), split across the two HWDGE queues (sync + scalar).
  - cc stage: division by exp(ln-ln) (which ping-ponged ACT table sets,
    85 loads x 1.3us) replaced by an int16-magic + 1 Newton-step
    reciprocal on DVE; only Square remains on ACT -> one table set.
    Final accumulation via scalar_tensor_tensor accum_out (no ACT pass).
  - f0/f1 drained into one [96,1920] tile; cc ops run once per out-slice
    on [96,2,192]-strided views (halves DVE/ACT per-op overhead).
  - D-diff: dB pair moved to the (otherwise idle) GPSIMD engine.

Numerically validated in numpy (bf16 inputs, bf16 snapshots, Newton
reciprocal): rel err ~1.5e-5 vs f32 reference.
"""

import sys

import numpy as np

sys.path.insert(0, "/opt/trn_rl_repo")

import contextlib

import concourse.bacc as bacc
import concourse.mybir as mybir
from concourse import tile
from concourse.bass_utils import run_bass_kernel_spmd

F32 = mybir.dt.float32
BF16 = mybir.dt.bfloat16
I16 = mybir.dt.int16
AOT = mybir.AluOpType
ACTF = mybir.ActivationFunctionType
AXL = mybir.AxisListType

H = 192
W = 192
D_TOT = 192
HE = 200   # extended h (4 raw-zero pad each side)
WE = 200   # extended w
PAD = 4
N_CORES = 8

HA = 112   # H-pass out: ext rows 4..115  == orig h 0..111
HB = 80    # H-pass out: ext rows 116..195 == orig h 112..191
KT = 128   # chanT partitions: ext-h 0..127
KB = 88    # chanB partitions: ext-h 112..199

BAND_C = 1.0 / 27.0
NCH = 5
FREE = NCH * WE            # 1000 (channel tiles, snapshots)
PIECE = 500                # free elems per matmul (PSUM: placed at 512 offs)
TPIECE = 480

EPS = float(np.finfo(np.float32).eps)
MAGIC = 0x7EF0             # bf16 reciprocal seed: bits(r0) = MAGIC - bits(x)


def _band(rows, cols, lo, hi, val):
    k = np.arange(rows)[:, None]
    m = np.arange(cols)[None, :]
    return np.where((k - m >= lo) & (k - m <= hi), val, 0.0).astype(np.float32)


def make_consts():
    import ml_dtypes

    # master upper band, k-m in [0,8]; sliced for all four matmul uses
    return _band(120, 112, 0, 8, BAND_C).astype(ml_dtypes.bfloat16)


def build_program(din, dout):
    assert din == dout + 2 * PAD
    nc = bacc.Bacc(
        "TRN2", target_bir_lowering=False, debug=False, num_devices=N_CORES
    )

    xin_d = nc.dram_tensor("xin", [din, HE, 2 * WE], BF16, kind="ExternalInput")
    band_d = nc.dram_tensor("band", [120, 112], BF16, kind="ExternalInput")
    out_d = nc.dram_tensor("out", [96, 1], F32, kind="ExternalOutput")

    xin = xin_d.ap()

    with tile.TileContext(nc) as tc, contextlib.ExitStack() as ctx:
        consts = ctx.enter_context(tc.tile_pool(name="consts", bufs=1))
        bigx = ctx.enter_context(tc.tile_pool(name="bigx", bufs=1))
        chans = ctx.enter_context(tc.tile_pool(name="chans", bufs=3))
        snaps = ctx.enter_context(tc.tile_pool(name="snaps", bufs=11))
        diffs = ctx.enter_context(tc.tile_pool(name="diffs", bufs=2))
        tts = ctx.enter_context(tc.tile_pool(name="tts", bufs=2))
        ffs = ctx.enter_context(tc.tile_pool(name="ffs", bufs=2))
        ccs = ctx.enter_context(tc.tile_pool(name="ccs", bufs=2))
        accp = ctx.enter_context(tc.tile_pool(name="accp", bufs=1))
        ps_h = ctx.enter_context(tc.tile_pool(name="psh", bufs=1, space="PSUM"))
        ps_w = ctx.enter_context(tc.tile_pool(name="psw", bufs=1, space="PSUM"))

        band = consts.tile([120, 112], BF16, tag="band")
        nc.sync.dma_start(band[:], band_d.ap())

        bias_nh = consts.tile([128, 1], F32, tag="bias_nh")
        nc.vector.memset(bias_nh[:], -0.5)

        # resident input: rows 0..127 (T) and 112..199 (B), one z-slice =
        # 400 bf16 per partition ([0:200]=targ, [200:400]=pred)
        XT = bigx.tile([KT, din * 2 * WE], BF16, tag="XT")
        XB = bigx.tile([KB, din * 2 * WE], BF16, tag="XB")
        XT3 = XT.rearrange("p (z w) -> p z w", z=din)
        XB3 = XB.rearrange("p (z w) -> p z w", z=din)
        CH = 8  # slices per input-load chunk
        for c in range(din // CH):
            z0 = c * CH
            src = xin[z0 : z0 + CH, :, :]
            nc.sync.dma_start(
                XT3[:, z0 : z0 + CH, :],
                src[:, 0:KT, :].rearrange("z h w -> h z w"),
            )
            nc.sync.dma_start(
                XB3[:, z0 : z0 + CH, :],
                src[:, HE - KB : HE, :].rearrange("z h w -> h z w"),
            )

        # H-cum PSUM; free padded to 1024 so each 500-piece sits in one bank
        psA = ps_h.tile([HA, 1024], F32, tag="psA")
        psB = ps_h.tile([HB, 1024], F32, tag="psB")
        psA3 = psA.rearrange("p (b w) -> p b w", b=2)  # [*, 2, 512]
        psB3 = psB.rearrange("p (b w) -> p b w", b=2)

        zsnapA = consts.tile([HA, FREE], BF16, tag="zsnapA")
        zsnapB = consts.tile([HB, FREE], BF16, tag="zsnapB")
        nc.vector.memset(zsnapA[:], 0.0)
        nc.vector.memset(zsnapB[:], 0.0)

        acc = accp.tile([96, dout], F32, tag="acc")
        nc.vector.memset(acc[:], 0.0)

        # Persistent ping-pong diff tiles; free layout [wc:2][ch:5][128] where
        # cols 0..103 of each 128-block hold ext-w 0..103 (wc0) / 96..199
        # (wc1) and cols 104..127 stay zero (memset once) so the x-bar
        # transposes read fully-initialized 128-wide blocks.
        diff_tiles = []
        for pp in range(2):
            dA = diffs.tile(
                [HA, 2 * NCH * 128], BF16, tag=f"dA{pp}", name=f"dA{pp}"
            )
            dB = diffs.tile(
                [HB, 2 * NCH * 128], BF16, tag=f"dB{pp}", name=f"dB{pp}"
            )
            nc.vector.memset(dA[:], 0.0)
            nc.vector.memset(dB[:], 0.0)
            diff_tiles.append((dA, dB))

        snapsA = {}
        snapsB = {}

        def prep_pair(z):
            # channels for slices z, z+1 in one tile pair (halved op count)
            chanT = chans.tile([KT, 2 * FREE], BF16, tag="chanT", name="chanT")
            chanB = chans.tile([KB, 2 * FREE], BF16, tag="chanB", name="chanB")
            for ch, X3 in ((chanT, XT3), (chanB, XB3)):
                np_ = ch.shape[0]
                raw = X3[0:np_, z : z + 2, :]   # [np_, 2, 400]
                c3 = ch.rearrange("p (z f) -> p z f", z=2)
                # ch0 = I-0.5, ch1 = J-0.5
                nc.vector.tensor_scalar_add(c3[:, :, 0 : 2 * WE], raw, -0.5)
                # ch2 = (I-0.5)^2, ch3 = (J-0.5)^2
                nc.scalar.activation(
                    c3[:, :, 2 * WE : 4 * WE], raw, ACTF.Square,
                    bias=bias_nh[0:np_, :],
                )
                # ch4 = (J-0.5)*(I-0.5)
                nc.vector.scalar_tensor_tensor(
                    c3[:, :, 4 * WE : FREE],
                    raw[:, :, WE : 2 * WE],
                    -0.5,
                    c3[:, :, 0:WE],
                    AOT.add,
                    AOT.mult,
                )
            return chanT, chanB

        def h_pass(z, chanT, chanB, zi):
            # start only on the first slice (PSUM then accumulates across
            # slices = cumsum over D). stop is a HW no-op; asserting it every
            # slice keeps the simulator's PSUM-read-while-group-open check
            # happy, with skip_group_check for the reopen.
            start = z == 0
            off = zi * FREE
            # A pieces back-to-back (shared lhsT), then B
            for p in range(2):
                sl = slice(off + p * PIECE, off + (p + 1) * PIECE)
                nc.tensor.matmul(
                    psA3[:, p, 0:PIECE], band[0:120, 0:HA], chanT[0:120, sl],
                    start=start, stop=True, skip_group_check=True,
                )
            for p in range(2):
                sl = slice(off + p * PIECE, off + (p + 1) * PIECE)
                nc.tensor.matmul(
                    psB3[:, p, 0:PIECE], band[0:KB, 0:HB], chanB[:, sl],
                    start=start, stop=True, skip_group_check=True,
                )

            sA = snaps.tile([HA, FREE], BF16, tag="snapA", name="snapA")
            sB = snaps.tile([HB, FREE], BF16, tag="snapB", name="snapB")
            sA3 = sA.rearrange("p (b w) -> p b w", b=2)
            sB3 = sB.rearrange("p (b w) -> p b w", b=2)
            nc.vector.tensor_copy(sA3[:, 0:1, :], psA3[:, 0:1, 0:PIECE])
            nc.scalar.copy(sA3[:, 1:2, :], psA3[:, 1:2, 0:PIECE])
            nc.scalar.copy(sB3[:], psB3[:, :, 0:PIECE])
            snapsA[z] = sA
            snapsB[z] = sB

        def w_pass(oz):
            hi_A, hi_B = snapsA[oz + 8], snapsB[oz + 8]
            lo_A = zsnapA if oz == 0 else snapsA[oz - 1]
            lo_B = zsnapB if oz == 0 else snapsB[oz - 1]
            snapsA.pop(oz - 2, None)
            snapsB.pop(oz - 2, None)

            # D-filtered slice into the ping-pong diff tiles (valid cols
            # 0..103 per block: wc0 = ext-w 0..103, wc1 = ext-w 96..199).
            # dA pair on DVE, dB pair on GPSIMD (otherwise idle).
            dA, dB = diff_tiles[oz % 2]
            for dd, hi, lo, eng in (
                (dA, hi_A, lo_A, nc.vector),
                (dB, hi_B, lo_B, nc.vector),
            ):
                d3 = dd.rearrange("p (b c w) -> p b c w", b=2, c=NCH)
                hi3 = hi.rearrange("p (c w) -> p c w", c=NCH)
                lo3 = lo.rearrange("p (c w) -> p c w", c=NCH)
                for wc in range(2):
                    w0 = wc * 96
                    eng.tensor_tensor(
                        d3[:, wc, :, 0:104],
                        hi3[:, :, w0 : w0 + 104],
                        lo3[:, :, w0 : w0 + 104],
                        AOT.subtract,
                    )

            # batched x-bar transposes: all 10 (wc,ch) 128-blocks of each
            # diff tile in ONE instruction; block b lands at free offset
            # b*192 (+0 for A-rows 0..111, +112 for B-rows 112..191).
            tt = tts.tile([128, 2 * NCH * H], BF16, tag="tt", name="tt")
            tt3 = tt.rearrange("p (b h) -> p b h", b=2 * NCH)
            nc.sync.dma_start_transpose(tt3[:, :, 0:HA], dA[:])
            nc.sync.dma_start_transpose(tt3[:, :, HA:H], dB[:])

            pw0 = ps_w.tile([96, 1024], F32, tag="pw0", name="pw0")
            pw1 = ps_w.tile([96, 1024], F32, tag="pw1", name="pw1")
            pw03 = pw0.rearrange("p (b w) -> p b w", b=2)
            pw13 = pw1.rearrange("p (b w) -> p b w", b=2)
            for p in range(2):
                sl = slice(p * TPIECE, (p + 1) * TPIECE)
                sl1 = slice(NCH * H + p * TPIECE, NCH * H + (p + 1) * TPIECE)
                nc.tensor.matmul(
                    pw03[:, p, 0:TPIECE], band[0:104, 0:96], tt[0:104, sl],
                    start=True, stop=True,
                )
                nc.tensor.matmul(
                    pw13[:, p, 0:TPIECE], band[0:104, 0:96], tt[0:104, sl1],
                    start=True, stop=True,
                )

            # drain both wc halves into ONE [96, 1920] tile; cc ops then run
            # once per out-slice on [96, 2, 192]-strided 3D views.
            ff = ffs.tile([96, 2 * NCH * H], BF16, tag="ff", name="ff")
            ff3 = ff.rearrange("p (b w) -> p b w", b=2)  # [96, 2, 960]
            nc.scalar.copy(
                ff3[:, 0:1, :].rearrange("p o (b w) -> p (o b) w", b=2),
                pw03[:, :, 0:TPIECE],
            )
            nc.scalar.copy(
                ff3[:, 1:2, :].rearrange("p o (b w) -> p (o b) w", b=2),
                pw13[:, :, 0:TPIECE],
            )

            F_I = ff3[:, :, 0:H]
            F_J = ff3[:, :, H : 2 * H]
            F_SQ = ff3[:, :, 0 : 2 * H]          # [I, J] pair
            F_CONV = ff3[:, :, 2 * H : 4 * H]    # [conv_I2, conv_J2]
            F_IJ = ff3[:, :, 4 * H : 5 * H]

            sc = ccs.tile([96, 2 * 1152], BF16, tag="sc", name="sc")
            sc3 = sc.rearrange("p (b w) -> p b w", b=2)
            t1v = sc3[:, :, 0:H]
            s12 = sc3[:, :, H : 2 * H]
            sqs = sc3[:, :, 2 * H : 4 * H]
            sg = sc3[:, :, 4 * H : 6 * H]
            sg1 = sc3[:, :, 4 * H : 5 * H]
            sg2 = sc3[:, :, 5 * H : 6 * H]
            scd = ccs.tile([96, 2 * 960], BF16, tag="scd", name="scd")
            scd3 = scd.rearrange("p (b w) -> p b w", b=2)
            den = scd3[:, :, 0:H]
            r0 = scd3[:, :, H : 2 * H]
            tq = scd3[:, :, 2 * H : 3 * H]
            r1n = scd3[:, :, 3 * H : 4 * H]
            s2f = scd3[:, :, 4 * H : 5 * H]
            den2 = t1v   # t1v dead after s12
            ccout = tq   # tq dead after r1n

            nc.vector.tensor_tensor(t1v, F_I, F_J, AOT.mult)
            nc.vector.tensor_tensor(s12, F_IJ, t1v, AOT.subtract)
            nc.scalar.activation(sqs, F_SQ, ACTF.Square)
            nc.vector.tensor_tensor(sg, F_CONV, sqs, AOT.subtract)
            nc.vector.tensor_tensor(den, sg1, sg2, AOT.mult)
            nc.vector.tensor_scalar_max(den2, den, EPS)
            # reciprocal seed: bits(r0) = MAGIC - bits(den2)
            nc.vector.tensor_scalar(
                r0.bitcast(I16), den2.bitcast(I16), -1, MAGIC,
                AOT.mult, AOT.add,
            )
            # one Newton step, sign-folded: r1n = (den2*r0 - 2)*r0 = -recip
            nc.vector.tensor_tensor(tq, den2, r0, AOT.mult)
            nc.vector.scalar_tensor_tensor(
                r1n, tq, 2.0, r0, AOT.subtract, AOT.mult
            )
            nc.scalar.activation(s2f, s12, ACTF.Square)
            # cc = (-s2f) * r1n = s12^2 * recip(den), accumulated into acc
            nc.vector.scalar_tensor_tensor(
                ccout, s2f, -1.0, r1n, AOT.mult, AOT.mult,
                accum_out=acc[:, oz : oz + 1],
            )

        for z0 in range(0, din, 2):
            chanT, chanB = prep_pair(z0)
            for zi in range(2):
                z = z0 + zi
                h_pass(z, chanT, chanB, zi)
                oz = z - 8
                if 0 <= oz < dout:
                    w_pass(oz)

        accv = accp.tile([96, 1], F32, tag="accv")
        nc.vector.tensor_reduce(accv[:], acc[:], AXL.X, AOT.add)
        nc.sync.dma_start(out_d.ap(), accv[:])

    nc.compile()
    return nc


_PROGRAM_CACHE = {}


def _get_program(din, dout):
    key = (din, dout)
    if key not in _PROGRAM_CACHE:
        _PROGRAM_CACHE[key] = build_program(din, dout)
    return _PROGRAM_CACHE[key]


def make_in_maps(pred, target):
    import ml_dtypes

    pred = np.asarray(pred).reshape(D_TOT, H, W).astype(np.float32)
    targ = np.asarray(target).reshape(D_TOT, H, W).astype(np.float32)

    dout = D_TOT // N_CORES
    din = dout + 2 * PAD

    # one interleaved, padded, bf16 volume: [D+8, 200, 400]
    big = np.zeros((D_TOT + 2 * PAD, HE, 2 * WE), ml_dtypes.bfloat16)
    big[PAD:-PAD, PAD : PAD + H, PAD : PAD + W] = targ
    big[PAD:-PAD, PAD : PAD + H, WE + PAD : WE + PAD + W] = pred

    band = make_consts()
    in_maps = []
    for c in range(N_CORES):
        s = c * dout
        in_maps.append(
            {
                "xin": np.ascontiguousarray(big[s : s + din]),
                "band": band,
            }
        )
    return in_maps, din, dout


def kernel(pred, target):
    in_maps, din, dout = make_in_maps(pred, target)
    nc = _get_program(din, dout)
    res = run_bass_kernel_spmd(nc, in_maps, core_ids=list(range(N_CORES)))
    total = sum(float(r["out"].astype(np.float64).sum()) for r in res.results)
    return np.float32(1.0 - total / float(D_TOT * H * W))
